# revision 30
# baseline (speedup 1.0000x reference)
"""Attention w/ KV cache on 8 trn2 NeuronCores.

Sharding: core i handles (batch b = i//2, query-half = i%2).  Each core
computes full attention for 512 query rows of one batch against that
batch's 5120 keys (4096 cache + 1024 new).  No collectives: the output
projection contracts over all heads, which every core has for its rows.

All tensors are kept feature-major ("transposed") on chip so that every
matmul consumes operands in its natural orientation; the required
transposes (x^T, cache-K^T, partition-major V) are done host-side in
numpy inside kernel().  Matmuls run in float32r (full PE rate for free
dim >= 256), accumulation in fp32 PSUM.

Perf structure:
- S-matmul pairs are (jt, jt+16): the two K-cache halves live on SBUF
  partitions 0-63 / 64-127, so the pair occupies disjoint PE row groups
  and streams concurrently.  New-token tiles get the same treatment via
  a mirrored second half (kTnM).
- exp is split between ScalarE (exact) and VectorE (Schraudolph int
  trick) so neither activation engine is the pipeline limiter.
- Phase-1 DMAs are chunked and spread across queues; projections use
  contraction-outer accumulation groups so the PE starts as soon as the
  first chunks land.
- Per-head softmax normalization is deferred by one head so its
  reciprocal->broadcast-matmul chain never stalls the PE.
"""

import sys

sys.path.insert(0, "/opt/trn_rl_repo")

import numpy as np

B, N, C, H, D = 4, 1024, 1024, 16, 64
LC = 4096          # cached keys
L = LC + N         # total keys
NH = N // 2        # query rows per core
NCORES = 8
SCALE = float(D) ** -0.5
CT = C // 128      # contraction tiles (8)
QM = C // 128      # q output col-tiles (8)
NT = N // 128      # new-token l-tiles (8)
LT = L // 128      # total l-tiles (40)
KCT = LC // 128    # cached l-tiles (32)

# Schraudolph exp: exp(x) ~= bitcast_f32(int32(EXPA * x + EXPB)).
# The DVE computes (x * EXPA + EXPB) in fp32 and truncates on the i32
# convert; EXPC recenters the sawtooth error (calibrated numerically).
EXPA = float(2 ** 23) / float(np.log(2.0))
EXPC = 430000.0
EXPB = float(127 * 2 ** 23) - EXPC
# which of the 20 pair-steps run their exp on the vector engine
DVE_PAIRS = (1, 4, 7, 10, 13, 16, 18)
USE_GPSIMD_CAST = False   # Schraudolph rounding copy on GpSimd instead of DVE
AV_LAG = 2                # AV pairs issue this many steps behind S pairs
USE_APPROX_RECIP = True   # reciprocal_approx_fast + ACT re-round vs DVE reciprocal

_CACHE = {}


def _build(reps: int = 1, dve_pairs=DVE_PAIRS):
    import concourse.mybir as mybir
    import concourse.tile as tile
    from concourse import bacc
    from contextlib import ExitStack

    f32 = mybir.dt.float32
    f32r = mybir.dt.float32r
    i32 = mybir.dt.int32
    AF = mybir.ActivationFunctionType
    MUL = mybir.AluOpType.mult
    ADD = mybir.AluOpType.add

    nc = bacc.Bacc("TRN2", target_bir_lowering=False, debug=False)

    xT_d = nc.dram_tensor("xT", [C, N], f32r, kind="ExternalInput").ap()
    kT_d = nc.dram_tensor("kT", [H, 2, D, LC // 2], f32r, kind="ExternalInput").ap()
    v_d = nc.dram_tensor("v", [H, 128, D + 1, KCT], f32r, kind="ExternalInput").ap()
    # q/k weight columns pre-blocked host-side: [m, p, ct*128] so a whole
    # strip is 4KB-contiguous per partition (one descriptor per partition)
    wqk_d = nc.dram_tensor("wqk", [2 * QM, 128, CT * 128], f32r, kind="ExternalInput").ap()
    wv_d = nc.dram_tensor("wv", [C, C], f32r, kind="ExternalInput").ap()
    wproj_d = nc.dram_tensor("wproj", [QM, 128, CT * 128], f32r, kind="ExternalInput").ap()
    bias_d = nc.dram_tensor("bias", [128, QM], f32, kind="ExternalInput").ap()
    yT_d = nc.dram_tensor("yT", [C, NH], f32, kind="ExternalOutput").ap()

    dve_set = set(dve_pairs)

    with tile.TileContext(nc) as tc:
      # reps>1 runs the body under a hardware loop: the NEFF holds ONE copy
      # of the body, so multi-rep builds stay rep-1-sized on disk and the
      # reps-slope timing isolates device time.
      with ExitStack() as loop_es:
       if reps > 1:
        loop_es.enter_context(tc.For_i(0, reps, 1))
       with ExitStack() as es:
        pp = es.enter_context(tc.tile_pool(name="persist", bufs=1))
        qT = pp.tile([128, QM, NH], f32r)            # q^T, head h at (64*(h%2), h//2)
        qd2 = pp.tile([128, QM, NH], f32r)           # q^T copies on opposite halves
        kTn = pp.tile([128, QM, N], f32r)            # new-K^T, same packing
        kTnM = pp.tile([128, QM, NH], f32r)          # mirror of kTn[:, :, 512:] halves
        vn = pp.tile([128, NT, H, D + 1], f32r)      # new-V + ones col, l-partition-major
        aoT = pp.tile([128, CT, NH], f32r)           # normalized attention out^T
        ones = pp.tile([1, 64], f32r)
        bias = pp.tile([128, QM], f32)
        ones32 = pp.tile([128, 128], f32)
        scratch = pp.tile([1, 1], f32)

        nc.vector.memset(ones32[:], 1.0)
        # dummy exp: forces the ACT table load at t=0, hidden under DMA
        nc.scalar.activation(scratch[:], ones32[0:1, 0:1], AF.Exp)
        nc.vector.tensor_copy(ones[:], ones32[0:1, 0:64])
        nc.vector.tensor_copy(
            vn[:, :, :, D], ones32[:].rearrange("p (a b) -> p a b", a=NT)
        )
        nc.gpsimd.dma_start(bias[:], bias_d[:])

        # kv-cache tile pools live across phases so head 0 prefetches early
        kcp = es.enter_context(tc.tile_pool(name="kc", bufs=2))
        vcp = es.enter_context(tc.tile_pool(name="vc", bufs=2))
        kcs, vcs = {}, {}

        def load_kc(h):
            kc = kcp.tile([128, KCT // 2, 128], f32r, tag="kc", name=f"kc{h}")
            nc.gpsimd.dma_start(
                kc[:].rearrange("p t j -> p (t j)"),
                kT_d[h].rearrange("a d l -> (a d) l"),
            )
            kcs[h] = kc

        def load_vc(h):
            vc = vcp.tile([128, D + 1, KCT], f32r, tag="vc", name=f"vc{h}")
            nc.gpsimd.dma_start(vc[:], v_d[h])
            vcs[h] = vc

        load_kc(0)

        # ---------------- phase 1: projections ----------------
        # xp scope spans q/k-proj + v-proj, then frees its 32KB for phase 2
        ph1 = ExitStack()
        xp = ph1.enter_context(tc.tile_pool(name="xp", bufs=1))
        xT = xp.tile([128, CT, N], f32r)
        # chunked x^T load on the sync queue (critical path)
        for ct in range(CT):
            nc.sync.dma_start(xT[:, ct, :], xT_d[ct * 128 : (ct + 1) * 128, :])

        with tc.tile_pool(name="w1", bufs=3) as wpool, \
             tc.tile_pool(name="ps1", bufs=8, space="PSUM") as ps1:
            # q/k weight strips on the scalar queue (parallel to x^T);
            # contraction-outer groups so MMs start on the first chunks
            groups = [[0, 1, 2], [3, 4, 5], [6, 7]] + [
                [m, m + 1] for m in range(QM, 2 * QM, 2)
            ]
            wtiles = {}
            for group in groups:
                nch = 1 if group[0] < QM else 2
                for m in group:
                    w = wpool.tile([128, CT, 128], f32r, tag="w", name=f"w{m}")
                    if m < 3:
                        # first group chunked per-ct so the PE starts on
                        # chunk (m,0) ~64KB in instead of a 512KB strip
                        for ct in range(CT):
                            nc.scalar.dma_start(
                                w[:, ct, :],
                                wqk_d[m, :, ct * 128 : (ct + 1) * 128],
                            )
                    else:
                        nc.scalar.dma_start(
                            w[:].rearrange("p t c -> p (t c)"), wqk_d[m]
                        )
                    wtiles[m] = w
                psums = {
                    (gm, j): ps1.tile(
                        [128, 512], f32, tag="ps1", name=f"ps1_{gm}_{j}"
                    )
                    for gm in group
                    for j in range(nch)
                }
                for ct in range(CT):
                    for gm in group:
                        for j in range(nch):
                            nc.tensor.matmul(
                                psums[(gm, j)][:],
                                lhsT=wtiles[gm][:, ct, :],
                                rhs=xT[:, ct, j * 512 : (j + 1) * 512],
                                start=(ct == 0),
                                stop=(ct == CT - 1),
                                tile_position=(0, 0),
                            )
                for gm in group:
                    for j in range(nch):
                        if gm < QM:
                            nc.vector.tensor_copy(
                                qT[:, gm, :], psums[(gm, j)][:]
                            )
                            # opposite-half q copies via SBUF->SBUF DMA
                            nc.gpsimd.dma_start(
                                qd2[64:128, gm, :], qT[0:64, gm, :]
                            )
                            nc.gpsimd.dma_start(
                                qd2[0:64, gm, :], qT[64:128, gm, :]
                            )
                        else:
                            nc.vector.tensor_copy(
                                kTn[:, gm - QM, j * 512 : (j + 1) * 512],
                                psums[(gm, j)][:],
                            )
            # mirror the second half of new-K onto the opposite partitions
            nc.gpsimd.dma_start(kTnM[0:64, :, :], kTn[64:128, :, 512:1024])
            nc.gpsimd.dma_start(kTnM[64:128, :, :], kTn[0:64, :, 512:1024])

        # v-cache for head 0 deferred here to fit SBUF during the w-strip
        # phase; the DMA still completes well before phase 2
        load_vc(0)

        # new-V projection: wv streamed per-chunk, two nt-halves
        with tc.tile_pool(name="wv", bufs=3) as wvp, \
             tc.tile_pool(name="psv", bufs=8, space="PSUM") as psv:
            for half in range(2):
                nts = range(half * 4, half * 4 + 4)
                psums = {
                    (nt, j): psv.tile(
                        [128, 512], f32, tag="psv", name=f"psv_{nt}_{j}"
                    )
                    for nt in nts
                    for j in range(2)
                }
                for ct in range(CT):
                    wv = wvp.tile([128, N], f32r, tag="wv", name=f"wv_{half}_{ct}")
                    nc.sync.dma_start(
                        wv[:], wv_d[ct * 128 : (ct + 1) * 128, :]
                    )
                    for nt in nts:
                        for j in range(2):
                            nc.tensor.matmul(
                                psums[(nt, j)][:],
                                lhsT=xT[:, ct, nt * 128 : (nt + 1) * 128],
                                rhs=wv[:, j * 512 : (j + 1) * 512],
                                start=(ct == 0),
                                stop=(ct == CT - 1),
                                tile_position=(0, 0),
                            )
                for nt in nts:
                    for j in range(2):
                        nc.vector.tensor_copy(
                            vn[:, nt, j * 8 : (j + 1) * 8, 0:D],
                            psums[(nt, j)][:].rearrange("p (h d) -> p h d", d=D),
                        )
        ph1.close()  # frees xT's SBUF before phase-2 pools open

        # ---------------- phase 2 + 3 ----------------
        with tc.tile_pool(name="at", bufs=3) as atp, \
             tc.tile_pool(name="u1", bufs=2) as u1p, \
             tc.tile_pool(name="nrm", bufs=1) as nrmp, \
             tc.tile_pool(name="sps", bufs=2, space="PSUM") as sps, \
             tc.tile_pool(name="ops", bufs=2, space="PSUM") as ops, \
             tc.tile_pool(name="bps", bufs=2, space="PSUM") as bps, \
             tc.tile_pool(name="w3", bufs=4) as wp3, \
             tc.tile_pool(name="ysb", bufs=2) as ysb:

            # pair steps: cache tiles (jt, jt+16) sit on opposite SBUF
            # halves -> disjoint PE row groups -> concurrent matmuls.
            # New tiles (32+j, 36+j) ditto via the kTnM mirror.
            pairs = [(j, j + KCT // 2) for j in range(KCT // 2)]
            pairs += [(KCT + j, KCT + 4 + j) for j in range(4)]

            ophist = {}
            w3tiles = []

            def load_w3(m):
                w = wp3.tile([128, CT, 128], f32r, tag="w3", name=f"w3_{m}")
                nc.sync.dma_start(
                    w[:].rearrange("p t c -> p (t c)"), wproj_d[m]
                )
                w3tiles.append(w)

            def norm_early(h):
                # reciprocal of the ones-row sums; emitted one head later
                # so it never gates the PE.  approx_fast is one DVE pass
                # (~18 correct bits) vs the 8-cycle iterative divide; the
                # ACT Copy re-rounds to f32r for the broadcast matmul
                # (Copy is in the resident exp table set - no swap).
                op = ophist[h]
                rc = nrmp.tile([1, NH], f32r, tag="rc", name=f"rc{h}")
                if USE_APPROX_RECIP:
                    # approx_fast reads garbage from PSUM on HW - stage the
                    # sums row into SBUF first.  raw row 64 = sums copy,
                    # row 0 = recip out (base partitions must be 0/32/64/96;
                    # separate tile because the verifier tracks f32r
                    # rounding per memory location)
                    raw = nrmp.tile([65, NH], f32, tag="rcraw", name=f"rcraw{h}")
                    nc.vector.tensor_copy(raw[64:65, :], op[64:65, :])
                    nc.vector.reciprocal_approx_fast(raw[0:1, :], raw[64:65, :])
                    nc.scalar.activation(rc[:], raw[0:1, :], AF.Copy)
                else:
                    with nc.allow_low_precision(reason="fp32r recip is fp32-width"):
                        nc.vector.reciprocal(rc[:], op[64:65, :])
                return rc[:]

            def norm_late(h, rc):
                hm, hp = h // 2, (h % 2) * 64
                op = ophist.pop(h)
                bp = bps.tile([64, NH], f32, tag="bp", name=f"bp{h}")
                nc.tensor.matmul(
                    bp[:], lhsT=ones[:], rhs=rc, start=True, stop=True,  # rc is an AP
                    tile_position=(0, 0),
                )
                bc = nrmp.tile([64, NH], f32r, tag="bc", name=f"bc{h}")
                nc.vector.tensor_copy(bc[:], bp[:])
                if hp == 0:
                    nc.vector.tensor_tensor(
                        aoT[0:64, hm, :], op[0:64, :], bc[:], op=MUL,
                    )
                else:
                    tmp = nrmp.tile([64, NH], f32r, tag="tmp", name=f"tmp{h}")
                    nc.vector.tensor_tensor(tmp[:], op[0:64, :], bc[:], op=MUL)
                    nc.sync.dma_start(aoT[64:128, hm, :], tmp[:])

            rc_prev = None
            for h in range(H):
                hm, hp = h // 2, (h % 2) * 64
                if h + 1 < H:
                    load_kc(h + 1)
                    load_vc(h + 1)
                if h == H - 3:
                    load_w3(0)
                    load_w3(1)
                elif h == H - 2:
                    load_w3(2)
                elif h == H - 1:
                    load_w3(3)
                kc, vc = kcs.pop(h), vcs.pop(h)
                if h > 0:
                    rc_prev = norm_early(h - 1)

                op = ops.tile([128, NH], f32, tag="op", name=f"op{h}")
                ophist[h] = op

                def kparts(jt):
                    if jt < KCT // 2:
                        return kc[0:64, jt, :], 0
                    if jt < KCT:
                        return kc[64:128, jt - KCT // 2, :], 64
                    j = jt - KCT
                    if j < 4:
                        return kTn[hp : hp + 64, hm, j * 128 : (j + 1) * 128], hp
                    b2 = 64 - hp
                    return kTnM[b2 : b2 + 64, hm, (j - 4) * 128 : (j - 3) * 128], b2

                # AV pairs issue AV_LAG steps behind their S pairs so the
                # PE FIFO never blocks on an exp in flight: while exp(i)
                # runs on ACT/DVE, the PE streams S(i+1), S(i+2) and the
                # AVs of earlier steps.
                pend = []

                def flush_av():
                    fpi, fjts, fat = pend.pop(0)
                    for s, jt in enumerate(fjts):
                        vt = vc[:, :, jt] if jt < KCT else vn[:, jt - KCT, h, :]
                        nc.tensor.matmul(
                            op[0:65, :], lhsT=vt, rhs=fat[:, s * NH : (s + 1) * NH],
                            start=(fpi == 0 and s == 0),
                            stop=(fpi == len(pairs) - 1 and s == 1),
                            tile_position=(0, 0), skip_group_check=True,
                        )

                for pi, (jt1, jt2) in enumerate(pairs):
                    sp = sps.tile([128, 2 * NH], f32, tag="sp")
                    for s, jt in enumerate((jt1, jt2)):
                        lhsT, base = kparts(jt)
                        rq = (qT if base == hp else qd2)[base : base + 64, hm, :]
                        nc.tensor.matmul(
                            sp[:, s * NH : (s + 1) * NH], lhsT=lhsT, rhs=rq,
                            start=True, stop=True, tile_position=(base, 0),
                        )
                    at = atp.tile([128, 2 * NH], f32r, tag="at")
                    # head 0: DVE is still draining phase-1 copies; keep
                    # its exps on ACT so the pipeline fills without stalls
                    if pi in dve_set and h > 0:
                        # Schraudolph exp: DVE int write, then a rounding
                        # copy (f32r matmul inputs must come from one)
                        u1 = u1p.tile([128, 2 * NH], i32, tag="u1")
                        nc.vector.tensor_scalar(
                            u1[:], sp[:], EXPA, EXPB, MUL, ADD
                        )
                        if USE_GPSIMD_CAST:
                            nc.gpsimd.tensor_copy(at[:], u1[:].bitcast(f32))
                        else:
                            nc.vector.tensor_copy(at[:], u1[:].bitcast(f32))
                    else:
                        nc.scalar.activation(at[:], sp[:], AF.Exp)
                    pend.append((pi, (jt1, jt2), at))
                    if len(pend) > AV_LAG:
                        flush_av()
                while pend:
                    flush_av()

                if h > 0:
                    norm_late(h - 1, rc_prev)

            rc_last = norm_early(H - 1)
            norm_late(H - 1, rc_last)

            # ---------------- phase 3: output projection ----------------
            for m in range(QM):
                if m + 4 < QM:
                    load_w3(m + 4)
                w = w3tiles[m]
                yp = ops.tile([128, NH], f32, tag="op", name=f"yp{m}")
                for ct in range(CT):
                    nc.tensor.matmul(
                        yp[:], lhsT=w[:, ct, :], rhs=aoT[:, ct, :],
                        start=(ct == 0), stop=(ct == CT - 1), tile_position=(0, 0),
                    )
                y = ysb.tile([128, NH], f32, tag="y")
                nc.vector.tensor_scalar_add(y[:], yp[:], bias[:, m : m + 1])
                nc.sync.dma_start(yT_d[m * 128 : (m + 1) * 128, :], y[:])

    nc.compile()
    return nc


def get_nc(reps: int = 1):
    key = f"nc{reps}"
    if key not in _CACHE:
        _CACHE[key] = _build(reps)
    return _CACHE[key]


def make_inputs(x, kv_cache, w_qkv, w_proj, b_proj):
    """Host-side shard + layout prep.  Returns list of 8 input maps."""
    x = np.ascontiguousarray(x, dtype=np.float32)
    kv_cache = np.ascontiguousarray(kv_cache, dtype=np.float32)
    w_qkv = np.ascontiguousarray(w_qkv, dtype=np.float32)
    w_proj = np.ascontiguousarray(w_proj, dtype=np.float32)
    b_proj = np.ascontiguousarray(b_proj, dtype=np.float32)

    # fold the softmax 1/sqrt(D) into the q projection columns (exact:
    # SCALE is a power of two)
    w_qkv = w_qkv.copy()
    w_qkv[:, :C] *= SCALE

    # block q/k weight columns [m, p, ct*128]: strip DMAs become one
    # 4KB-contiguous descriptor per partition
    wqk_blk = np.ascontiguousarray(
        w_qkv[:, : 2 * C]
        .reshape(CT, 128, 2 * QM, 128)
        .transpose(2, 1, 0, 3)
        .reshape(2 * QM, 128, CT * 128)
    )
    wv_cols = np.ascontiguousarray(w_qkv[:, 2 * C :])
    wproj_blk = np.ascontiguousarray(
        w_proj.reshape(CT, 128, QM, 128)
        .transpose(2, 1, 0, 3)
        .reshape(QM, 128, CT * 128)
    )

    bias_h = np.ascontiguousarray(b_proj.reshape(QM, 128).T)
    in_maps = []
    for core in range(NCORES):
        b, half = core // 2, core % 2
        xb = x[b]                                    # [N, C]
        own = xb[half * NH : (half + 1) * NH]
        other = xb[(1 - half) * NH : (2 - half) * NH]
        xrot = np.concatenate([own, other], axis=0)  # rotated: own half first
        xT = np.ascontiguousarray(xrot.T)            # [C, N]
        kT = np.ascontiguousarray(
            kv_cache[0, b].reshape(H, 2, LC // 2, D).transpose(0, 1, 3, 2)
        )                                            # [H, 2, D, LC//2]
        v = np.empty((H, 128, D + 1, KCT), dtype=np.float32)
        v[:, :, :D, :] = kv_cache[1, b].reshape(H, KCT, 128, D).transpose(0, 2, 3, 1)
        v[:, :, D, :] = 1.0
        in_maps.append(
            {
                "xT": xT,
                "kT": kT,
                "v": v,
                "wqk": wqk_blk,
                "wv": wv_cols,
                "wproj": wproj_blk,
                "bias": bias_h,
            }
        )
    return in_maps


def assemble(results):
    y = np.empty((B, N, C), dtype=np.float32)
    for core in range(NCORES):
        b, half = core // 2, core % 2
        y[b, half * NH : (half + 1) * NH] = results[core]["yT"].T
    return y


class _Runner:
    """Persistent jitted SPMD executor (mirrors bass2jax.run_bass_via_pjrt but
    caches the jitted callable so repeated kernel() calls skip re-tracing)."""

    def __init__(self, nc, n_cores):
        import jax
        from jax.sharding import Mesh, PartitionSpec
        from jax.experimental.shard_map import shard_map
        import concourse.mybir as mybir
        from concourse import bass2jax

        bass2jax.install_neuronx_cc_hook()
        self.n_cores = n_cores
        pid_name = nc.partition_id_tensor.name if nc.partition_id_tensor else None
        in_names, out_names, out_avals, zero_outs = [], [], [], []
        for alloc in nc.m.functions[0].allocations:
            if not isinstance(alloc, mybir.MemoryLocationSet):
                continue
            name = alloc.memorylocations[0].name
            if alloc.kind == "ExternalInput":
                if name != pid_name:
                    in_names.append(name)
            elif alloc.kind == "ExternalOutput":
                out_names.append(name)
                shape = tuple(alloc.tensor_shape)
                dtype = mybir.dt.np(alloc.dtype)
                out_avals.append(jax.core.ShapedArray(shape, dtype))
                zero_outs.append(np.zeros(shape, dtype))
        self.in_names, self.out_names = in_names, out_names
        self.out_avals, self.zero_outs = out_avals, zero_outs
        n_params, n_outs = len(in_names), len(out_names)
        all_names = list(in_names + out_names)
        if pid_name is not None:
            all_names.append(pid_name)
        all_names = tuple(all_names)

        def _body(*args):
            operands = list(args)
            if pid_name is not None:
                operands.append(bass2jax.partition_id_tensor())
            return tuple(
                bass2jax._bass_exec_p.bind(
                    *operands,
                    out_avals=tuple(out_avals),
                    in_names=all_names,
                    out_names=tuple(out_names),
                    lowering_input_output_aliases=(),
                    sim_require_finite=True,
                    sim_require_nnan=True,
                    nc=nc,
                )
            )

        devices = jax.devices()[:n_cores]
        self.mesh = Mesh(np.asarray(devices), ("core",))
        in_specs = (PartitionSpec("core"),) * (n_params + n_outs)
        out_specs = (PartitionSpec("core"),) * n_outs
        self.fn = jax.jit(
            shard_map(
                _body,
                mesh=self.mesh,
                in_specs=in_specs,
                out_specs=out_specs,
                check_rep=False,
            ),
            keep_unused=True,
        )

    def __call__(self, in_maps):
        import jax

        args = [
            np.concatenate([np.asarray(m[name]) for m in in_maps], axis=0)
            for name in self.in_names
        ]
        args += [
            np.zeros((self.n_cores * z.shape[0], *z.shape[1:]), z.dtype)
            for z in self.zero_outs
        ]
        outs = self.fn(*args)
        jax.block_until_ready(outs)
        return [
            {
                name: np.asarray(outs[i]).reshape(
                    self.n_cores, *self.out_avals[i].shape
                )[c]
                for i, name in enumerate(self.out_names)
            }
            for c in range(self.n_cores)
        ]


def _get_runner():
    if "runner" not in _CACHE:
        _CACHE["runner"] = _Runner(get_nc(), NCORES)
    return _CACHE["runner"]


def kernel(x, kv_cache, w_qkv, w_proj, b_proj):
    in_maps = make_inputs(x, kv_cache, w_qkv, w_proj, b_proj)
    try:
        results = _get_runner()(in_maps)
    except Exception:
        import traceback

        traceback.print_exc()
        from concourse.bass_utils import run_bass_kernel_spmd

        results = run_bass_kernel_spmd(get_nc(), in_maps, list(range(NCORES))).results
    return assemble(results)



# revision 32
# speedup vs baseline: 1.2039x; 1.2039x over previous
"""Attention w/ KV cache on 8 trn2 NeuronCores.

Sharding: core i handles (batch b = i//2, query-half = i%2).  Each core
computes full attention for 512 query rows of one batch against that
batch's 5120 keys (4096 cache + 1024 new).  No collectives: the output
projection contracts over all heads, which every core has for its rows.

All tensors are kept feature-major ("transposed") on chip so that every
matmul consumes operands in its natural orientation; the required
transposes (x^T, cache-K^T, partition-major V) are done host-side in
numpy inside kernel().  Matmuls run in float32r (full PE rate for free
dim >= 256), accumulation in fp32 PSUM.

Perf structure:
- S-matmul pairs are (jt, jt+16): the two K-cache halves live on SBUF
  partitions 0-63 / 64-127, so the pair occupies disjoint PE row groups
  and streams concurrently.  New-token tiles get the same treatment via
  a mirrored second half (kTnM).
- exp is split between ScalarE (exact) and VectorE (Schraudolph int
  trick) so neither activation engine is the pipeline limiter.
- Phase-1 DMAs are chunked and spread across queues; projections use
  contraction-outer accumulation groups so the PE starts as soon as the
  first chunks land.
- Per-head softmax normalization is deferred by one head so its
  reciprocal->broadcast-matmul chain never stalls the PE.
"""

import sys

sys.path.insert(0, "/opt/trn_rl_repo")

import numpy as np

B, N, C, H, D = 4, 1024, 1024, 16, 64
LC = 4096          # cached keys
L = LC + N         # total keys
NH = N // 2        # query rows per core
NCORES = 8
SCALE = float(D) ** -0.5
CT = C // 128      # contraction tiles (8)
QM = C // 128      # q output col-tiles (8)
NT = N // 128      # new-token l-tiles (8)
LT = L // 128      # total l-tiles (40)
KCT = LC // 128    # cached l-tiles (32)

# Schraudolph exp: exp(x) ~= bitcast_f32(int32(EXPA * x + EXPB)).
# The DVE computes (x * EXPA + EXPB) in fp32 and truncates on the i32
# convert; EXPC recenters the sawtooth error (calibrated numerically).
EXPA = float(2 ** 23) / float(np.log(2.0))
EXPC = 430000.0
EXPB = float(127 * 2 ** 23) - EXPC
# which of the 20 pair-steps run their exp on the vector engine
DVE_PAIRS = (1, 4, 7, 10, 13, 16, 18)
USE_GPSIMD_CAST = False   # Schraudolph rounding copy on GpSimd instead of DVE
AV_LAG = 2                # AV pairs issue this many steps behind S pairs
USE_APPROX_RECIP = True   # reciprocal_approx_fast + ACT re-round vs DVE reciprocal

_CACHE = {}


def _build(reps: int = 1, dve_pairs=DVE_PAIRS):
    import concourse.mybir as mybir
    import concourse.tile as tile
    from concourse import bacc
    from contextlib import ExitStack

    f32 = mybir.dt.float32
    f32r = mybir.dt.float32r
    i32 = mybir.dt.int32
    AF = mybir.ActivationFunctionType
    MUL = mybir.AluOpType.mult
    ADD = mybir.AluOpType.add

    nc = bacc.Bacc("TRN2", target_bir_lowering=False, debug=False)

    xT_d = nc.dram_tensor("xT", [C, N], f32r, kind="ExternalInput").ap()
    kT_d = nc.dram_tensor("kT", [H, 2, D, LC // 2], f32r, kind="ExternalInput").ap()
    v_d = nc.dram_tensor("v", [H, 128, D + 1, KCT], f32r, kind="ExternalInput").ap()
    # q/k weight columns pre-blocked host-side: [m, p, ct*128] so a whole
    # strip is 4KB-contiguous per partition (one descriptor per partition)
    wqk_d = nc.dram_tensor("wqk", [2 * QM, 128, CT * 128], f32r, kind="ExternalInput").ap()
    wv_d = nc.dram_tensor("wv", [C, C], f32r, kind="ExternalInput").ap()
    wproj_d = nc.dram_tensor("wproj", [QM, 128, CT * 128], f32r, kind="ExternalInput").ap()
    bias_d = nc.dram_tensor("bias", [128, QM], f32, kind="ExternalInput").ap()
    yT_d = nc.dram_tensor("yT", [C, NH], f32, kind="ExternalOutput").ap()

    dve_set = set(dve_pairs)

    with tile.TileContext(nc) as tc:
      # reps>1 runs the body under a hardware loop: the NEFF holds ONE copy
      # of the body, so multi-rep builds stay rep-1-sized on disk and the
      # reps-slope timing isolates device time.
      with ExitStack() as loop_es:
       if reps > 1:
        loop_es.enter_context(tc.For_i(0, reps, 1))
       with ExitStack() as es:
        pp = es.enter_context(tc.tile_pool(name="persist", bufs=1))
        qT = pp.tile([128, QM, NH], f32r)            # q^T, head h at (64*(h%2), h//2)
        qd2 = pp.tile([128, QM, NH], f32r)           # q^T copies on opposite halves
        kTn = pp.tile([128, QM, N], f32r)            # new-K^T, same packing
        kTnM = pp.tile([128, QM, NH], f32r)          # mirror of kTn[:, :, 512:] halves
        vn = pp.tile([128, NT, H, D + 1], f32r)      # new-V + ones col, l-partition-major
        aoT = pp.tile([128, CT, NH], f32r)           # normalized attention out^T
        ones = pp.tile([1, 64], f32r)
        bias = pp.tile([128, QM], f32)
        ones32 = pp.tile([128, 128], f32)
        scratch = pp.tile([1, 1], f32)

        nc.vector.memset(ones32[:], 1.0)
        # dummy exp: forces the ACT table load at t=0, hidden under DMA
        nc.scalar.activation(scratch[:], ones32[0:1, 0:1], AF.Exp)
        nc.vector.tensor_copy(ones[:], ones32[0:1, 0:64])
        nc.vector.tensor_copy(
            vn[:, :, :, D], ones32[:].rearrange("p (a b) -> p a b", a=NT)
        )
        nc.gpsimd.dma_start(bias[:], bias_d[:])

        # kv-cache tile pools live across phases so head 0 prefetches early
        kcp = es.enter_context(tc.tile_pool(name="kc", bufs=2))
        vcp = es.enter_context(tc.tile_pool(name="vc", bufs=2))
        kcs, vcs = {}, {}

        def load_kc(h):
            kc = kcp.tile([128, KCT // 2, 128], f32r, tag="kc", name=f"kc{h}")
            nc.gpsimd.dma_start(
                kc[:].rearrange("p t j -> p (t j)"),
                kT_d[h].rearrange("a d l -> (a d) l"),
            )
            kcs[h] = kc

        def load_vc(h):
            vc = vcp.tile([128, D + 1, KCT], f32r, tag="vc", name=f"vc{h}")
            nc.gpsimd.dma_start(vc[:], v_d[h])
            vcs[h] = vc

        load_kc(0)

        # ---------------- phase 1: projections ----------------
        # xp scope spans q/k-proj + v-proj, then frees its 32KB for phase 2
        ph1 = ExitStack()
        xp = ph1.enter_context(tc.tile_pool(name="xp", bufs=1))
        xT = xp.tile([128, CT, N], f32r)
        # chunked x^T load on the sync queue (critical path)
        for ct in range(CT):
            nc.sync.dma_start(xT[:, ct, :], xT_d[ct * 128 : (ct + 1) * 128, :])

        with tc.tile_pool(name="w1", bufs=3) as wpool, \
             tc.tile_pool(name="ps1", bufs=8, space="PSUM") as ps1:
            # q/k weight strips on the scalar queue (parallel to x^T);
            # contraction-outer groups so MMs start on the first chunks
            groups = [[0, 1, 2], [3, 4, 5], [6, 7]] + [
                [m, m + 1] for m in range(QM, 2 * QM, 2)
            ]
            wtiles = {}
            for group in groups:
                nch = 1 if group[0] < QM else 2
                for m in group:
                    w = wpool.tile([128, CT, 128], f32r, tag="w", name=f"w{m}")
                    if m < 3:
                        # first group chunked per-ct so the PE starts on
                        # chunk (m,0) ~64KB in instead of a 512KB strip
                        for ct in range(CT):
                            nc.scalar.dma_start(
                                w[:, ct, :],
                                wqk_d[m, :, ct * 128 : (ct + 1) * 128],
                            )
                    else:
                        nc.scalar.dma_start(
                            w[:].rearrange("p t c -> p (t c)"), wqk_d[m]
                        )
                    wtiles[m] = w
                psums = {
                    (gm, j): ps1.tile(
                        [128, 512], f32, tag="ps1", name=f"ps1_{gm}_{j}"
                    )
                    for gm in group
                    for j in range(nch)
                }
                for ct in range(CT):
                    for gm in group:
                        for j in range(nch):
                            nc.tensor.matmul(
                                psums[(gm, j)][:],
                                lhsT=wtiles[gm][:, ct, :],
                                rhs=xT[:, ct, j * 512 : (j + 1) * 512],
                                start=(ct == 0),
                                stop=(ct == CT - 1),
                                tile_position=(0, 0),
                            )
                for gm in group:
                    for j in range(nch):
                        if gm < QM:
                            nc.vector.tensor_copy(
                                qT[:, gm, :], psums[(gm, j)][:]
                            )
                            # opposite-half q copies via SBUF->SBUF DMA
                            nc.gpsimd.dma_start(
                                qd2[64:128, gm, :], qT[0:64, gm, :]
                            )
                            nc.gpsimd.dma_start(
                                qd2[0:64, gm, :], qT[64:128, gm, :]
                            )
                        else:
                            nc.vector.tensor_copy(
                                kTn[:, gm - QM, j * 512 : (j + 1) * 512],
                                psums[(gm, j)][:],
                            )
            # mirror the second half of new-K onto the opposite partitions
            nc.gpsimd.dma_start(kTnM[0:64, :, :], kTn[64:128, :, 512:1024])
            nc.gpsimd.dma_start(kTnM[64:128, :, :], kTn[0:64, :, 512:1024])

        # v-cache for head 0 deferred here to fit SBUF during the w-strip
        # phase; the DMA still completes well before phase 2
        load_vc(0)

        # new-V projection: wv streamed per-chunk, two nt-halves
        with tc.tile_pool(name="wv", bufs=3) as wvp, \
             tc.tile_pool(name="psv", bufs=8, space="PSUM") as psv:
            for half in range(2):
                nts = range(half * 4, half * 4 + 4)
                psums = {
                    (nt, j): psv.tile(
                        [128, 512], f32, tag="psv", name=f"psv_{nt}_{j}"
                    )
                    for nt in nts
                    for j in range(2)
                }
                for ct in range(CT):
                    wv = wvp.tile([128, N], f32r, tag="wv", name=f"wv_{half}_{ct}")
                    nc.sync.dma_start(
                        wv[:], wv_d[ct * 128 : (ct + 1) * 128, :]
                    )
                    for nt in nts:
                        for j in range(2):
                            nc.tensor.matmul(
                                psums[(nt, j)][:],
                                lhsT=xT[:, ct, nt * 128 : (nt + 1) * 128],
                                rhs=wv[:, j * 512 : (j + 1) * 512],
                                start=(ct == 0),
                                stop=(ct == CT - 1),
                                tile_position=(0, 0),
                            )
                for nt in nts:
                    for j in range(2):
                        nc.vector.tensor_copy(
                            vn[:, nt, j * 8 : (j + 1) * 8, 0:D],
                            psums[(nt, j)][:].rearrange("p (h d) -> p h d", d=D),
                        )
        ph1.close()  # frees xT's SBUF before phase-2 pools open

        # ---------------- phase 2 + 3 ----------------
        with tc.tile_pool(name="at", bufs=3) as atp, \
             tc.tile_pool(name="u1", bufs=2 if USE_GPSIMD_CAST else 1) as u1p, \
             tc.tile_pool(name="nrm", bufs=1) as nrmp, \
             tc.tile_pool(name="sps", bufs=2, space="PSUM") as sps, \
             tc.tile_pool(name="ops", bufs=2, space="PSUM") as ops, \
             tc.tile_pool(name="bps", bufs=2, space="PSUM") as bps, \
             tc.tile_pool(name="w3", bufs=4) as wp3, \
             tc.tile_pool(name="ysb", bufs=2) as ysb:

            # pair steps: cache tiles (jt, jt+16) sit on opposite SBUF
            # halves -> disjoint PE row groups -> concurrent matmuls.
            # New tiles (32+j, 36+j) ditto via the kTnM mirror.
            pairs = [(j, j + KCT // 2) for j in range(KCT // 2)]
            pairs += [(KCT + j, KCT + 4 + j) for j in range(4)]

            ophist = {}
            w3tiles = []

            def load_w3(m):
                w = wp3.tile([128, CT, 128], f32r, tag="w3", name=f"w3_{m}")
                nc.sync.dma_start(
                    w[:].rearrange("p t c -> p (t c)"), wproj_d[m]
                )
                w3tiles.append(w)

            def norm_early(h):
                # reciprocal of the ones-row sums; emitted one head later
                # so it never gates the PE.  approx_fast is one DVE pass
                # (~18 correct bits) vs the 8-cycle iterative divide; the
                # ACT Copy re-rounds to f32r for the broadcast matmul
                # (Copy is in the resident exp table set - no swap).
                op = ophist[h]
                rc = nrmp.tile([1, NH], f32r, tag="rc", name=f"rc{h}")
                if USE_APPROX_RECIP:
                    # approx_fast is HW-correct ONLY for base partition 0
                    # and SBUF operands (PSUM or partition-64 inputs read
                    # garbage) - stage the sums at partition 0 and use
                    # free-dim offsets; separate tile from rc because the
                    # verifier tracks f32r rounding per memory location
                    raw = nrmp.tile([1, 2 * NH], f32, tag="rcraw", name=f"rcraw{h}")
                    nc.vector.tensor_copy(raw[0:1, 0:NH], op[64:65, :])
                    nc.vector.reciprocal_approx_fast(
                        raw[0:1, NH : 2 * NH], raw[0:1, 0:NH]
                    )
                    nc.scalar.activation(rc[:], raw[0:1, NH : 2 * NH], AF.Copy)
                else:
                    with nc.allow_low_precision(reason="fp32r recip is fp32-width"):
                        nc.vector.reciprocal(rc[:], op[64:65, :])
                return rc[:]

            def norm_late(h, rc):
                hm, hp = h // 2, (h % 2) * 64
                op = ophist.pop(h)
                bp = bps.tile([64, NH], f32, tag="bp", name=f"bp{h}")
                nc.tensor.matmul(
                    bp[:], lhsT=ones[:], rhs=rc, start=True, stop=True,  # rc is an AP
                    tile_position=(0, 0),
                )
                bc = nrmp.tile([64, NH], f32r, tag="bc", name=f"bc{h}")
                nc.vector.tensor_copy(bc[:], bp[:])
                if hp == 0:
                    nc.vector.tensor_tensor(
                        aoT[0:64, hm, :], op[0:64, :], bc[:], op=MUL,
                    )
                else:
                    tmp = nrmp.tile([64, NH], f32r, tag="tmp", name=f"tmp{h}")
                    nc.vector.tensor_tensor(tmp[:], op[0:64, :], bc[:], op=MUL)
                    nc.sync.dma_start(aoT[64:128, hm, :], tmp[:])

            rc_prev = None
            for h in range(H):
                hm, hp = h // 2, (h % 2) * 64
                if h + 1 < H:
                    load_kc(h + 1)
                    load_vc(h + 1)
                if h == H - 3:
                    load_w3(0)
                    load_w3(1)
                elif h == H - 2:
                    load_w3(2)
                elif h == H - 1:
                    load_w3(3)
                kc, vc = kcs.pop(h), vcs.pop(h)
                if h > 0:
                    rc_prev = norm_early(h - 1)

                op = ops.tile([128, NH], f32, tag="op", name=f"op{h}")
                ophist[h] = op

                def kparts(jt):
                    if jt < KCT // 2:
                        return kc[0:64, jt, :], 0
                    if jt < KCT:
                        return kc[64:128, jt - KCT // 2, :], 64
                    j = jt - KCT
                    if j < 4:
                        return kTn[hp : hp + 64, hm, j * 128 : (j + 1) * 128], hp
                    b2 = 64 - hp
                    return kTnM[b2 : b2 + 64, hm, (j - 4) * 128 : (j - 3) * 128], b2

                # AV pairs issue AV_LAG steps behind their S pairs so the
                # PE FIFO never blocks on an exp in flight: while exp(i)
                # runs on ACT/DVE, the PE streams S(i+1), S(i+2) and the
                # AVs of earlier steps.
                pend = []

                def flush_av():
                    fpi, fjts, fat = pend.pop(0)
                    for s, jt in enumerate(fjts):
                        vt = vc[:, :, jt] if jt < KCT else vn[:, jt - KCT, h, :]
                        nc.tensor.matmul(
                            op[0:65, :], lhsT=vt, rhs=fat[:, s * NH : (s + 1) * NH],
                            start=(fpi == 0 and s == 0),
                            stop=(fpi == len(pairs) - 1 and s == 1),
                            tile_position=(0, 0), skip_group_check=True,
                        )

                for pi, (jt1, jt2) in enumerate(pairs):
                    sp = sps.tile([128, 2 * NH], f32, tag="sp")
                    for s, jt in enumerate((jt1, jt2)):
                        lhsT, base = kparts(jt)
                        rq = (qT if base == hp else qd2)[base : base + 64, hm, :]
                        nc.tensor.matmul(
                            sp[:, s * NH : (s + 1) * NH], lhsT=lhsT, rhs=rq,
                            start=True, stop=True, tile_position=(base, 0),
                        )
                    at = atp.tile([128, 2 * NH], f32r, tag="at")
                    # head 0: DVE is still draining phase-1 copies; keep
                    # its exps on ACT so the pipeline fills without stalls
                    if pi in dve_set and h > 0:
                        # Schraudolph exp: DVE int write, then a rounding
                        # copy (f32r matmul inputs must come from one)
                        u1 = u1p.tile([128, 2 * NH], i32, tag="u1")
                        nc.vector.tensor_scalar(
                            u1[:], sp[:], EXPA, EXPB, MUL, ADD
                        )
                        if USE_GPSIMD_CAST:
                            nc.gpsimd.tensor_copy(at[:], u1[:].bitcast(f32))
                        else:
                            nc.vector.tensor_copy(at[:], u1[:].bitcast(f32))
                    else:
                        nc.scalar.activation(at[:], sp[:], AF.Exp)
                    pend.append((pi, (jt1, jt2), at))
                    if len(pend) > AV_LAG:
                        flush_av()
                while pend:
                    flush_av()

                if h > 0:
                    norm_late(h - 1, rc_prev)

            rc_last = norm_early(H - 1)
            norm_late(H - 1, rc_last)

            # ---------------- phase 3: output projection ----------------
            for m in range(QM):
                if m + 4 < QM:
                    load_w3(m + 4)
                w = w3tiles[m]
                yp = ops.tile([128, NH], f32, tag="op", name=f"yp{m}")
                for ct in range(CT):
                    nc.tensor.matmul(
                        yp[:], lhsT=w[:, ct, :], rhs=aoT[:, ct, :],
                        start=(ct == 0), stop=(ct == CT - 1), tile_position=(0, 0),
                    )
                y = ysb.tile([128, NH], f32, tag="y")
                nc.vector.tensor_scalar_add(y[:], yp[:], bias[:, m : m + 1])
                nc.sync.dma_start(yT_d[m * 128 : (m + 1) * 128, :], y[:])

    nc.compile()
    return nc


def get_nc(reps: int = 1):
    key = f"nc{reps}"
    if key not in _CACHE:
        _CACHE[key] = _build(reps)
    return _CACHE[key]


def make_inputs(x, kv_cache, w_qkv, w_proj, b_proj):
    """Host-side shard + layout prep.  Returns list of 8 input maps."""
    x = np.ascontiguousarray(x, dtype=np.float32)
    kv_cache = np.ascontiguousarray(kv_cache, dtype=np.float32)
    w_qkv = np.ascontiguousarray(w_qkv, dtype=np.float32)
    w_proj = np.ascontiguousarray(w_proj, dtype=np.float32)
    b_proj = np.ascontiguousarray(b_proj, dtype=np.float32)

    # fold the softmax 1/sqrt(D) into the q projection columns (exact:
    # SCALE is a power of two)
    w_qkv = w_qkv.copy()
    w_qkv[:, :C] *= SCALE

    # block q/k weight columns [m, p, ct*128]: strip DMAs become one
    # 4KB-contiguous descriptor per partition
    wqk_blk = np.ascontiguousarray(
        w_qkv[:, : 2 * C]
        .reshape(CT, 128, 2 * QM, 128)
        .transpose(2, 1, 0, 3)
        .reshape(2 * QM, 128, CT * 128)
    )
    wv_cols = np.ascontiguousarray(w_qkv[:, 2 * C :])
    wproj_blk = np.ascontiguousarray(
        w_proj.reshape(CT, 128, QM, 128)
        .transpose(2, 1, 0, 3)
        .reshape(QM, 128, CT * 128)
    )

    bias_h = np.ascontiguousarray(b_proj.reshape(QM, 128).T)
    in_maps = []
    for core in range(NCORES):
        b, half = core // 2, core % 2
        xb = x[b]                                    # [N, C]
        own = xb[half * NH : (half + 1) * NH]
        other = xb[(1 - half) * NH : (2 - half) * NH]
        xrot = np.concatenate([own, other], axis=0)  # rotated: own half first
        xT = np.ascontiguousarray(xrot.T)            # [C, N]
        kT = np.ascontiguousarray(
            kv_cache[0, b].reshape(H, 2, LC // 2, D).transpose(0, 1, 3, 2)
        )                                            # [H, 2, D, LC//2]
        v = np.empty((H, 128, D + 1, KCT), dtype=np.float32)
        v[:, :, :D, :] = kv_cache[1, b].reshape(H, KCT, 128, D).transpose(0, 2, 3, 1)
        v[:, :, D, :] = 1.0
        in_maps.append(
            {
                "xT": xT,
                "kT": kT,
                "v": v,
                "wqk": wqk_blk,
                "wv": wv_cols,
                "wproj": wproj_blk,
                "bias": bias_h,
            }
        )
    return in_maps


def assemble(results):
    y = np.empty((B, N, C), dtype=np.float32)
    for core in range(NCORES):
        b, half = core // 2, core % 2
        y[b, half * NH : (half + 1) * NH] = results[core]["yT"].T
    return y


class _Runner:
    """Persistent jitted SPMD executor (mirrors bass2jax.run_bass_via_pjrt but
    caches the jitted callable so repeated kernel() calls skip re-tracing)."""

    def __init__(self, nc, n_cores):
        import jax
        from jax.sharding import Mesh, PartitionSpec
        from jax.experimental.shard_map import shard_map
        import concourse.mybir as mybir
        from concourse import bass2jax

        bass2jax.install_neuronx_cc_hook()
        self.n_cores = n_cores
        pid_name = nc.partition_id_tensor.name if nc.partition_id_tensor else None
        in_names, out_names, out_avals, zero_outs = [], [], [], []
        for alloc in nc.m.functions[0].allocations:
            if not isinstance(alloc, mybir.MemoryLocationSet):
                continue
            name = alloc.memorylocations[0].name
            if alloc.kind == "ExternalInput":
                if name != pid_name:
                    in_names.append(name)
            elif alloc.kind == "ExternalOutput":
                out_names.append(name)
                shape = tuple(alloc.tensor_shape)
                dtype = mybir.dt.np(alloc.dtype)
                out_avals.append(jax.core.ShapedArray(shape, dtype))
                zero_outs.append(np.zeros(shape, dtype))
        self.in_names, self.out_names = in_names, out_names
        self.out_avals, self.zero_outs = out_avals, zero_outs
        n_params, n_outs = len(in_names), len(out_names)
        all_names = list(in_names + out_names)
        if pid_name is not None:
            all_names.append(pid_name)
        all_names = tuple(all_names)

        def _body(*args):
            operands = list(args)
            if pid_name is not None:
                operands.append(bass2jax.partition_id_tensor())
            return tuple(
                bass2jax._bass_exec_p.bind(
                    *operands,
                    out_avals=tuple(out_avals),
                    in_names=all_names,
                    out_names=tuple(out_names),
                    lowering_input_output_aliases=(),
                    sim_require_finite=True,
                    sim_require_nnan=True,
                    nc=nc,
                )
            )

        devices = jax.devices()[:n_cores]
        self.mesh = Mesh(np.asarray(devices), ("core",))
        in_specs = (PartitionSpec("core"),) * (n_params + n_outs)
        out_specs = (PartitionSpec("core"),) * n_outs
        self.fn = jax.jit(
            shard_map(
                _body,
                mesh=self.mesh,
                in_specs=in_specs,
                out_specs=out_specs,
                check_rep=False,
            ),
            keep_unused=True,
        )

    def __call__(self, in_maps):
        import jax

        args = [
            np.concatenate([np.asarray(m[name]) for m in in_maps], axis=0)
            for name in self.in_names
        ]
        args += [
            np.zeros((self.n_cores * z.shape[0], *z.shape[1:]), z.dtype)
            for z in self.zero_outs
        ]
        outs = self.fn(*args)
        jax.block_until_ready(outs)
        return [
            {
                name: np.asarray(outs[i]).reshape(
                    self.n_cores, *self.out_avals[i].shape
                )[c]
                for i, name in enumerate(self.out_names)
            }
            for c in range(self.n_cores)
        ]


def _get_runner():
    if "runner" not in _CACHE:
        _CACHE["runner"] = _Runner(get_nc(), NCORES)
    return _CACHE["runner"]


def kernel(x, kv_cache, w_qkv, w_proj, b_proj):
    in_maps = make_inputs(x, kv_cache, w_qkv, w_proj, b_proj)
    try:
        results = _get_runner()(in_maps)
    except Exception:
        import traceback

        traceback.print_exc()
        from concourse.bass_utils import run_bass_kernel_spmd

        results = run_bass_kernel_spmd(get_nc(), in_maps, list(range(NCORES))).results
    return assemble(results)

